# revision 6
# baseline (speedup 1.0000x reference)
"""AdaptiveJacobianPrunedViT on 8 TRN2 NeuronCores.

Strategy: data-parallel over batch (4 images/core). One NEFF runs the full
12-layer ViT with token pruning done as masking (the adaptive schedule only
drops ~1 token/layer, so full-size masked compute loses almost nothing).
Activations live in SBUF in transposed layout x^T [D=768 rows, 788 token
cols] so every GEMM's stationary operand is a plain weight slice and no
on-chip transposes are needed anywhere. Per-layer pruning stats are
partial-summed per core, AllReduced (~1KB), and the identical decision
(exact top-k tie semantics via rank computation) is replayed on all cores.

Numerics: f32 everywhere except attention weights A, values v, the MLP
hidden h and fc2 weights (bf16) — validated against the reference to keep
the data-dependent pruning schedule bit-identical.

kernel(**inputs) takes FULL unsharded inputs, returns [32, 1000] float32.
"""
import sys
if '/opt/trn_rl_repo' not in sys.path:
    sys.path.insert(0, '/opt/trn_rl_repo')

import contextlib

import numpy as np

import concourse.bass as bass
import concourse.tile as tile
from concourse import bacc, mybir

F32 = mybir.dt.float32
BF16 = mybir.dt.bfloat16
I32 = mybir.dt.int32
AF = mybir.ActivationFunctionType
ALU = mybir.AluOpType
AX = mybir.AxisListType

D = 768; NH = 12; DH = 64; DEPTH = 12; PATCH = 16; NCLS = 1000; DFF = 3072
GAMMA = 0.01; EPS = 1e-6
NPATCH = 196; TPI = 197
NCORES = 8; BATCH = 32; NI = BATCH // NCORES      # 4 images per core
TT = NI * TPI                                     # 788 tokens per core
KD = D // 128                                     # 6
MQK = (2 * D) // 128                              # 12
MFF = DFF // 128                                  # 24
CHUNKS = [(0, 128), (128, 69)]                    # per-image token chunks
PCHUNKS = [(0, 128), (128, 68)]                   # patch-space chunks (196)
NCH = [(0, 512), (512, 276)]                      # 788-wide free chunks
ZCH = [(0, 512), (512, 512), (1024, 512), (1536, 512), (2048, 316)]  # 2364

NLAYERS = DEPTH


def _mslice(w2d, m):
    """DRAM [K*128, Mtot] -> [128, nk, 128] lhsT m-tile view."""
    return w2d[:, m * 128:(m + 1) * 128].rearrange("(k p) c -> p k c", p=128)


def _col(vec_ap):
    """DRAM [n*128] -> [128, n] column-chunk view."""
    return vec_ap.rearrange("(d p) -> p d", p=128)


def build(nlayers=NLAYERS):
    nc = bacc.Bacc(target_bir_lowering=False)
    dr_ = {}

    def param(name, shape, dtype=F32):
        dr_[name] = nc.dram_tensor(name, list(shape), dtype, kind="ExternalInput")

    param("xpT", [D, TT]); param("posT", [D, TT]); param("embw", [D, D])
    param("ln1w", [DEPTH, D]); param("ln1b", [DEPTH, D])
    param("qkvw", [DEPTH, D, 3 * D]); param("qkvb", [DEPTH, 3 * D])
    param("vmw", [DEPTH, D, DH]); param("vmb", [DEPTH, DH])
    param("projw", [DEPTH, D, D]); param("projb", [DEPTH, D])
    param("ln2w", [DEPTH, D]); param("ln2b", [DEPTH, D])
    param("fc1w", [DEPTH, D, DFF]); param("fc1b", [DEPTH, DFF])
    param("fc2wb", [DEPTH, DFF, D], BF16); param("fc2b", [DEPTH, D])
    param("normw", [D]); param("normb", [D])
    param("headw", [D, NCLS]); param("headb", [NCLS])
    out_ext = nc.dram_tensor("out", [NI, NCLS], F32, kind="ExternalOutput")
    ap = {k: v.ap() for k, v in dr_.items()}

    with tile.TileContext(nc) as tc:
        Builder(tc, nc, ap, out_ext, nlayers).run()
    nc.finalize()
    return nc


class Builder:
    def __init__(self, tc, nc, ap, out_ext, nlayers):
        self.tc, self.nc, self.ap, self.out_ext = tc, nc, ap, out_ext
        self.nlayers = nlayers

    def run(self):
        tc = self.tc
        with contextlib.ExitStack() as ctx:
            P = {}
            for name, bufs, space in [
                ("cst", 1, "SBUF"), ("xp", 1, "SBUF"), ("b788", 6, "SBUF"),
                ("qk", 1, "SBUF"), ("vp", 1, "SBUF"), ("vmc", 1, "SBUF"),
                ("et", 1, "SBUF"), ("ht", 24, "SBUF"), ("wp6", 3, "SBUF"),
                ("wp24", 2, "SBUF"), ("vw", 1, "SBUF"), ("bc", 2, "SBUF"),
                ("sq", 2, "SBUF"), ("st", 1, "SBUF"), ("dec", 1, "SBUF"),
                ("bia", 1, "SBUF"),
                ("psA", 3, "PSUM"), ("psB", 2, "PSUM"), ("psR", 3, "PSUM"),
                ("dr", 2, "DRAM"),
            ]:
                P[name] = ctx.enter_context(
                    tc.tile_pool(name=name, bufs=bufs, space=space))
            self.P = P
            self._consts()
            self._embed()
            for l in range(self.nlayers):
                self._layer(l)
            self._head()

    # ---------------- constants / persistent state ----------------
    def _consts(self):
        nc, P = self.nc, self.P
        cst = P["cst"]
        self.ones_row = cst.tile([1, 788], F32, tag="ones_row")
        nc.vector.memset(self.ones_row, 1.0)
        self.ones_col = cst.tile([128, 1], F32, tag="ones_col")
        nc.vector.memset(self.ones_col, 1.0)
        self.ones_col_bf = cst.tile([128, 1], BF16, tag="ones_col_bf")
        nc.vector.memset(self.ones_col_bf, 1.0)
        self.eps_row = cst.tile([1, 1], F32, tag="eps_row")
        nc.vector.memset(self.eps_row, 1e-6)

        io_row_i = cst.tile([1, 196], I32, tag="io_row_i")
        nc.gpsimd.iota(io_row_i, pattern=[[1, 196]], base=0, channel_multiplier=0)
        io_row = cst.tile([1, 196], F32, tag="io_row")
        nc.vector.tensor_copy(io_row, io_row_i)
        io_rowb = cst.tile([128, 196], F32, tag="io_rowb")
        nc.gpsimd.partition_broadcast(io_rowb, io_row)
        io_col_i = cst.tile([128, 1], I32, tag="io_col_i")
        nc.gpsimd.iota(io_col_i, pattern=[[0, 1]], base=0, channel_multiplier=1)
        io_col = cst.tile([128, 1], F32, tag="io_col")
        nc.vector.tensor_copy(io_col, io_col_i)
        self.tri = []
        for c, (c0, csz) in enumerate(PCHUNKS):
            t = cst.tile([128, 196], F32, tag=f"tri{c}")
            col = cst.tile([128, 1], F32, tag=f"ioc{c}")
            nc.vector.tensor_scalar(col[:csz], io_col[:csz], float(c0), None,
                                    op0=ALU.add)
            nc.vector.tensor_scalar(t[:csz], io_rowb[:csz], col[:csz], None,
                                    op0=ALU.is_lt)
            self.tri.append(t)

        self.mask_prow = cst.tile([1, 196], F32, tag="mask_prow")
        nc.vector.memset(self.mask_prow, 1.0)
        self.mask_pcol = []
        for c in range(2):
            t = cst.tile([128, 1], F32, tag=f"mpc{c}")
            nc.vector.memset(t, 1.0)
            self.mask_pcol.append(t)
        self.pmask_tok, self.mbias_tok = [], []
        for c in range(2):
            t = cst.tile([128, 1], F32, tag=f"pmt{c}")
            nc.vector.memset(t, 1.0)
            if c == 0:
                nc.vector.memset(t[0:1], 0.0)
            self.pmask_tok.append(t)
            mb = cst.tile([128, 1], F32, tag=f"mbt{c}")
            nc.vector.memset(mb, 0.0)
            self.mbias_tok.append(mb)
        self.n_tok = cst.tile([1, 1], F32, tag="n_tok")
        nc.vector.memset(self.n_tok, float(NPATCH))
        self.prev_mass = cst.tile([1, 1], F32, tag="prev_mass")
        nc.vector.memset(self.prev_mass, 0.0)

    # ---------------- embed ----------------
    def _embed(self):
        nc, P, ap = self.nc, self.P, self.ap
        self.xT = [P["xp"].tile([128, 788], F32, tag=f"x{d}", name=f"xT{d}") for d in range(KD)]
        rhs = []
        for k in range(KD):
            t = P["vw"].tile([128, 788], F32, tag=f"vw{k}")
            nc.sync.dma_start(t, ap["xpT"][k * 128:(k + 1) * 128, :])
            rhs.append(t)
        for m in range(KD):
            wt = P["wp6"].tile([128, KD, 128], F32, tag="w")
            nc.sync.dma_start(wt, _mslice(ap["embw"], m))
            post = P["b788"].tile([128, 788], F32, tag="b788")
            nc.sync.dma_start(post, ap["posT"][m * 128:(m + 1) * 128, :])
            for n0, nsz in NCH:
                ps = P["psA"].tile([128, 512], F32, tag="psA")
                for k in range(KD):
                    nc.tensor.matmul(ps[:, :nsz], wt[:, k, :],
                                     rhs[k][:, n0:n0 + nsz],
                                     start=(k == 0), stop=(k == KD - 1))
                nc.vector.tensor_add(self.xT[m][:, n0:n0 + nsz], ps[:, :nsz],
                                     post[:, n0:n0 + nsz])

    # ---------------- transposed-layout layernorm ----------------
    def _ln_t(self, wrow_ap, brow_ap):
        nc, P = self.nc, self.P
        wcol = P["bia"].tile([128, KD], F32, tag="lnw")
        nc.sync.dma_start(wcol, _col(wrow_ap))
        bcol = P["bia"].tile([128, KD], F32, tag="lnb")
        nc.sync.dma_start(bcol, _col(brow_ap))

        mu = P["st"].tile([1, 788], F32, tag="ln_mu")
        msq = P["st"].tile([1, 788], F32, tag="ln_msq")
        for n0, nsz in NCH:
            ps_s = P["psR"].tile([1, 512], F32, tag="psR")
            ps_q = P["psR"].tile([1, 512], F32, tag="psR")
            for d in range(KD):
                sq = P["sq"].tile([128, 512], F32, tag="sq")
                nc.scalar.activation(sq[:, :nsz], self.xT[d][:, n0:n0 + nsz],
                                     AF.Square)
                nc.tensor.matmul(ps_s[:, :nsz], self.ones_col,
                                 self.xT[d][:, n0:n0 + nsz],
                                 start=(d == 0), stop=(d == KD - 1))
                nc.tensor.matmul(ps_q[:, :nsz], self.ones_col, sq[:, :nsz],
                                 start=(d == 0), stop=(d == KD - 1))
            nc.scalar.activation(mu[:, n0:n0 + nsz], ps_s[:, :nsz], AF.Copy,
                                 scale=1.0 / D)
            nc.scalar.activation(msq[:, n0:n0 + nsz], ps_q[:, :nsz], AF.Copy,
                                 scale=1.0 / D)
        tmp = P["st"].tile([1, 788], F32, tag="ln_tmp")
        nc.vector.tensor_mul(tmp, mu, mu)
        nc.vector.tensor_sub(tmp, msq, tmp)                     # var
        nc.scalar.activation(tmp, tmp, AF.Sqrt, bias=self.eps_row)
        nc.vector.reciprocal(tmp, tmp)                          # rstd
        crow = msq                                              # overlay (msq dead)
        nc.vector.tensor_mul(crow, mu, tmp)

        ab = P["bc"].tile([128, 788], F32, tag="bc")
        cb = P["bc"].tile([128, 788], F32, tag="bc")
        for row, dstt in ((tmp, ab), (crow, cb)):
            for n0, nsz in NCH:
                ps = P["psA"].tile([128, 512], F32, tag="psA")
                nc.tensor.matmul(ps[:, :nsz], self.ones_row[0:1, 0:128],
                                 row[0:1, n0:n0 + nsz], start=True, stop=True)
                nc.vector.tensor_copy(dstt[:, n0:n0 + nsz], ps[:, :nsz])

        outs = []
        for d in range(KD):
            o = P["b788"].tile([128, 788], F32, tag="b788")
            nc.vector.tensor_mul(o, self.xT[d], ab)
            nc.vector.tensor_sub(o, o, cb)
            nc.vector.tensor_scalar(o, o, wcol[:, d:d + 1], bcol[:, d:d + 1],
                                    op0=ALU.mult, op1=ALU.add)
            outs.append(o)
        return outs

    # ---------------- one transformer layer ----------------
    def _layer(self, l):
        nc, P, ap = self.nc, self.P, self.ap

        qkvb_col = P["bia"].tile([128, MQK], F32, tag="qkvb_col")
        nc.sync.dma_start(qkvb_col, _col(ap["qkvb"][l][0:2 * D]))
        fc1b_col = P["bia"].tile([128, MFF], F32, tag="fc1b_col")
        nc.sync.dma_start(fc1b_col, _col(ap["fc1b"][l]))
        vb = P["bia"].tile([1, D], F32, tag="vb")
        nc.sync.dma_start(vb, ap["qkvb"][l][None, 2 * D:])
        vmb = P["bia"].tile([1, DH], F32, tag="vmb")
        nc.sync.dma_start(vmb, ap["vmb"][l][None, :])
        projb = P["bia"].tile([1, D], F32, tag="projb")
        nc.sync.dma_start(projb, ap["projb"][l][None, :])
        fc2b = P["bia"].tile([1, D], F32, tag="fc2b")
        nc.sync.dma_start(fc2b, ap["fc2b"][l][None, :])

        xln = self._ln_t(ap["ln1w"][l], ap["ln1b"][l])

        # ---- q,k GEMM -> qkT[12] [128, 788] (q pre-scaled by 1/8 on host)
        qkt = [P["qk"].tile([128, 788], BF16, tag=f"qk{m}", name=f"qkt{m}") for m in range(MQK)]
        for m in range(MQK):
            wt = P["wp6"].tile([128, KD, 128], F32, tag="w")
            nc.sync.dma_start(wt, _mslice(ap["qkvw"][l], m))
            for n0, nsz in NCH:
                ps = P["psA"].tile([128, 512], F32, tag="psA")
                for k in range(KD):
                    nc.tensor.matmul(ps[:, :nsz], wt[:, k, :],
                                     xln[k][:, n0:n0 + nsz],
                                     start=(k == 0), stop=(k == KD - 1))
                nc.vector.tensor_scalar(qkt[m][:, n0:n0 + nsz], ps[:, :nsz],
                                        qkvb_col[:, m:m + 1], None, op0=ALU.add)

        # ---- v GEMM (bf16 out) + Vm GEMM (f32)
        vwt = []
        for k in range(KD):
            t = P["vw"].tile([128, 788], F32, tag=f"vw{k}")
            nc.sync.dma_start(t[:, :D], ap["qkvw"][l][k * 128:(k + 1) * 128, 2 * D:])
            vwt.append(t)
        vmwt = P["bia"].tile([128, KD, DH], F32, tag="vmw")
        nc.sync.dma_start(vmwt, ap["vmw"][l].rearrange("(k p) c -> p k c", p=128))

        vt = {}
        vmc = [P["vmc"].tile([128, NI, DH], F32, tag=f"vmc{c}", name=f"vmc{c}") for c in range(2)]
        for i in range(NI):
            for c, (c0, csz) in enumerate(CHUNKS):
                t = P["vp"].tile([128, D], BF16, tag=f"v{i}{c}")
                vt[(i, c)] = t
                g0 = i * TPI + c0
                for n0, nsz in [(0, 512), (512, 256)]:
                    ps = P["psA"].tile([128, 512], F32, tag="psA")
                    for k in range(KD):
                        nc.tensor.matmul(ps[:csz, :nsz], xln[k][:, g0:g0 + csz],
                                         vwt[k][:, n0:n0 + nsz],
                                         start=(k == 0), stop=False)
                    nc.tensor.matmul(ps[:csz, :nsz],
                                     self.ones_row[0:1, g0:g0 + csz],
                                     vb[0:1, n0:n0 + nsz], start=False, stop=True)
                    nc.vector.tensor_copy(t[:csz, n0:n0 + nsz], ps[:csz, :nsz])
                ps = P["psB"].tile([128, 512], F32, tag="psB")
                for k in range(KD):
                    nc.tensor.matmul(ps[:csz, :DH], xln[k][:, g0:g0 + csz],
                                     vmwt[:, k, :], start=(k == 0), stop=False)
                nc.tensor.matmul(ps[:csz, :DH], self.ones_row[0:1, g0:g0 + csz],
                                 vmb, start=False, stop=True)
                nc.vector.tensor_copy(vmc[c][:csz, i, :], ps[:csz, :DH])

        # ---- attention (per image), A in bf16
        attnT = [P["b788"].tile([128, 788], F32, tag="b788", name=f"attnT{l}_{p_}") for p_ in range(KD)]
        am0s = [P["st"].tile([128, NI, NH], F32, tag=f"am0s{c}", name=f"am0s{c}") for c in range(2)]
        for i in range(NI):
            g0 = i * TPI
            et = [P["et"].tile([128, NH * TPI], BF16, tag=f"et{c}", name=f"et{c}")
                  for c in range(2)]
            for h in range(NH):
                qt = qkt[h // 2]
                kt = qkt[6 + h // 2]
                off = (h % 2) * 64
                for c, (c0, csz) in enumerate(CHUNKS):
                    ps = P["psB"].tile([128, 512], F32, tag="psB")
                    nc.tensor.matmul(ps[:csz, :TPI],
                                     kt[off:off + 64, g0 + c0:g0 + c0 + csz],
                                     qt[off:off + 64, g0:g0 + TPI],
                                     start=True, stop=True)
                    nc.scalar.activation(et[c][:csz, h * TPI:(h + 1) * TPI],
                                         ps[:csz, :TPI], AF.Exp,
                                         bias=self.mbias_tok[c][:csz])
            for z0, zsz in ZCH:
                ps = P["psR"].tile([1, 512], F32, tag="psR")
                for c, (c0, csz) in enumerate(CHUNKS):
                    nc.tensor.matmul(ps[:, :zsz], self.ones_col_bf[:csz],
                                     et[c][:csz, z0:z0 + zsz],
                                     start=(c == 0), stop=(c == 1))
                zrow = P["st"].tile([1, 512], F32, tag="zrow")
                nc.vector.reciprocal(zrow[:, :zsz], ps[:, :zsz])
                psb = P["psA"].tile([128, 512], F32, tag="psA")
                nc.tensor.matmul(psb[:, :zsz], self.ones_row[0:1, 0:128],
                                 zrow[0:1, :zsz], start=True, stop=True)
                for c, (c0, csz) in enumerate(CHUNKS):
                    nc.vector.tensor_mul(et[c][:csz, z0:z0 + zsz],
                                         et[c][:csz, z0:z0 + zsz],
                                         psb[:csz, :zsz])
            for p in range(KD):
                ps = P["psB"].tile([128, 512], F32, tag="psB")
                for hh in range(2):
                    h = 2 * p + hh
                    r0 = hh * 64
                    for c, (c0, csz) in enumerate(CHUNKS):
                        nc.tensor.matmul(ps[r0:r0 + 64, :TPI],
                                         vt[(i, c)][:csz, h * DH:(h + 1) * DH],
                                         et[c][:csz, h * TPI:(h + 1) * TPI],
                                         start=(c == 0), stop=(c == 1))
                nc.vector.tensor_copy(attnT[p][:, g0:g0 + TPI], ps[:, :TPI])
            for c, (c0, csz) in enumerate(CHUNKS):
                ecol = et[c].rearrange("p (h t) -> p h t", h=NH)[:, :, 0]
                nc.vector.tensor_copy(am0s[c][:csz, i, :], ecol[:csz])

        # ---- proj + residual
        for m in range(KD):
            wt = P["wp6"].tile([128, KD, 128], F32, tag="w")
            nc.sync.dma_start(wt, _mslice(ap["projw"][l], m))
            for n0, nsz in NCH:
                ps = P["psA"].tile([128, 512], F32, tag="psA")
                for k in range(KD):
                    nc.tensor.matmul(ps[:, :nsz], wt[:, k, :],
                                     attnT[k][:, n0:n0 + nsz],
                                     start=(k == 0), stop=False)
                nc.tensor.matmul(ps[:, :nsz], projb[0:1, m * 128:(m + 1) * 128],
                                 self.ones_row[0:1, n0:n0 + nsz],
                                 start=False, stop=True)
                nc.vector.tensor_add(self.xT[m][:, n0:n0 + nsz], ps[:, :nsz],
                                     self.xT[m][:, n0:n0 + nsz])

        self._stats_and_decide(l, vmc, am0s)

        # ---- MLP (n-chunk outer; fc1 -> gelu(bf16 h) -> fc2(bf16) -> resid)
        xln2 = self._ln_t(ap["ln2w"][l], ap["ln2b"][l])
        for n0, nsz in NCH:
            hts = []
            for m in range(MFF):
                wt = P["wp6"].tile([128, KD, 128], F32, tag="w")
                nc.sync.dma_start(wt, _mslice(ap["fc1w"][l], m))
                ps = P["psA"].tile([128, 512], F32, tag="psA")
                for k in range(KD):
                    nc.tensor.matmul(ps[:, :nsz], wt[:, k, :],
                                     xln2[k][:, n0:n0 + nsz],
                                     start=(k == 0), stop=(k == KD - 1))
                ht = P["ht"].tile([128, 512], BF16, tag="ht")
                nc.scalar.activation(ht[:, :nsz], ps[:, :nsz], AF.Gelu,
                                     bias=fc1b_col[:, m:m + 1])
                hts.append(ht)
            for m2 in range(KD):
                wt = P["wp24"].tile([128, MFF, 128], BF16, tag="w24")
                nc.sync.dma_start(wt, _mslice(ap["fc2wb"][l], m2))
                ps = P["psA"].tile([128, 512], F32, tag="psA")
                for k in range(MFF):
                    nc.tensor.matmul(ps[:, :nsz], wt[:, k, :], hts[k][:, :nsz],
                                     start=(k == 0), stop=False)
                nc.tensor.matmul(ps[:, :nsz], fc2b[0:1, m2 * 128:(m2 + 1) * 128],
                                 self.ones_row[0:1, n0:n0 + nsz],
                                 start=False, stop=True)
                nc.vector.tensor_add(self.xT[m2][:, n0:n0 + nsz], ps[:, :nsz],
                                     self.xT[m2][:, n0:n0 + nsz])

    # ---------------- stats, AllReduce, pruning decision ----------------
    def _stats_and_decide(self, l, vmc, am0s):
        nc, P = self.nc, self.P
        if l == self.nlayers - 1:
            return
        st = P["st"]

        am0 = []
        for c, (c0, csz) in enumerate(CHUNKS):
            t = st.tile([128, NI], F32, tag=f"am0{c}")
            nc.vector.reduce_sum(t[:csz], am0s[c][:csz], axis=AX.X)
            nc.vector.tensor_scalar(t[:csz], t[:csz], 1.0 / NH, None, op0=ALU.mult)
            am0.append(t)

        recn = st.tile([1, 1], F32, tag="recn")
        nc.vector.reciprocal(recn, self.n_tok)

        psv = P["psR"].tile([1, 512], F32, tag="psR")
        for c, (c0, csz) in enumerate(CHUNKS):
            nc.tensor.matmul(psv[:, :NI * DH], self.pmask_tok[c][:csz],
                             vmc[c][:csz].rearrange("p a b -> p (a b)"),
                             start=(c == 0), stop=(c == 1))
        vmean = st.tile([1, NI * DH], F32, tag="vmean")
        nc.vector.tensor_scalar(vmean, psv[:, :NI * DH], recn, None, op0=ALU.mult)
        vmeanb = st.tile([128, NI * DH], F32, tag="vmeanb")
        nc.gpsimd.partition_broadcast(vmeanb, vmean)

        vn = []
        for c, (c0, csz) in enumerate(CHUNKS):
            vc = st.tile([128, NI * DH], F32, tag=f"vc{c}")
            nc.vector.tensor_sub(vc[:csz],
                                 vmc[c][:csz].rearrange("p a b -> p (a b)"),
                                 vmeanb[:csz])
            nc.vector.tensor_scalar(vc[:csz], vc[:csz],
                                    self.pmask_tok[c][:csz], None, op0=ALU.mult)
            nc.vector.tensor_mul(vc[:csz], vc[:csz], vc[:csz])
            t = st.tile([128, NI], F32, tag=f"vn{c}")
            nc.vector.reduce_sum(t[:csz],
                                 vc[:csz].rearrange("p (a b) -> p a b", a=NI),
                                 axis=AX.X)
            nc.scalar.activation(t[:csz], t[:csz], AF.Sqrt)
            vn.append(t)

        psm = P["psR"].tile([1, 512], F32, tag="psR")
        for c, (c0, csz) in enumerate(CHUNKS):
            nc.tensor.matmul(psm[:, :NI], self.pmask_tok[c][:csz], vn[c][:csz],
                             start=(c == 0), stop=(c == 1))
        mu = st.tile([1, NI], F32, tag="mu")
        nc.vector.tensor_scalar(mu, psm[:, :NI], recn, None, op0=ALU.mult)
        mub = st.tile([128, NI], F32, tag="mub")
        nc.gpsimd.partition_broadcast(mub, mu)
        dv = []
        psm2 = P["psR"].tile([1, 512], F32, tag="psR")
        for c, (c0, csz) in enumerate(CHUNKS):
            t = st.tile([128, NI], F32, tag=f"dv{c}")
            nc.vector.tensor_sub(t[:csz], vn[c][:csz], mub[:csz])
            dv.append(t)
            sqd = st.tile([128, NI], F32, tag=f"sqd{c}")
            nc.vector.tensor_mul(sqd[:csz], t[:csz], t[:csz])
            nc.vector.tensor_scalar(sqd[:csz], sqd[:csz],
                                    self.pmask_tok[c][:csz], None, op0=ALU.mult)
            nc.tensor.matmul(psm2[:, :NI], self.pmask_tok[c][:csz], sqd[:csz],
                             start=(c == 0), stop=(c == 1))
        recn1 = st.tile([1, 1], F32, tag="recn1")
        nc.vector.tensor_scalar(recn1, self.n_tok, 1.0, None, op0=ALU.subtract)
        nc.vector.reciprocal(recn1, recn1)
        sd = st.tile([1, NI], F32, tag="sd")
        nc.vector.tensor_scalar(sd, psm2[:, :NI], recn1, None, op0=ALU.mult)
        nc.scalar.activation(sd, sd, AF.Sqrt)
        nc.vector.tensor_scalar(sd, sd, EPS, None, op0=ALU.add)
        nc.vector.reciprocal(sd, sd)
        rsdb = st.tile([128, NI], F32, tag="rsdb")
        nc.gpsimd.partition_broadcast(rsdb, sd)

        jsum = []
        for c, (c0, csz) in enumerate(CHUNKS):
            j = st.tile([128, NI], F32, tag=f"j{c}")
            nc.vector.tensor_mul(j[:csz], dv[c][:csz], rsdb[:csz])
            nc.vector.tensor_scalar(j[:csz], j[:csz], 0.0, None, op0=ALU.max)
            nc.vector.tensor_mul(j[:csz], j[:csz], am0[c][:csz])
            nc.vector.tensor_scalar(j[:csz], j[:csz],
                                    self.pmask_tok[c][:csz], None, op0=ALU.mult)
            t = st.tile([128, 1], F32, tag=f"jsum{c}")
            nc.vector.reduce_sum(t[:csz], j[:csz], axis=AX.X)
            jsum.append(t)

        vm0 = st.tile([1, NI * DH], F32, tag="vm0")
        flat0 = vmc[0][0:1].rearrange("p a b -> p (a b)")
        nc.vector.tensor_mul(vm0, flat0, flat0)
        vm0n = st.tile([1, NI], F32, tag="vm0n")
        nc.vector.reduce_sum(vm0n, vm0.rearrange("p (a b) -> p a b", a=NI),
                             axis=AX.X)
        nc.scalar.activation(vm0n, vm0n, AF.Sqrt)
        rhoi = st.tile([1, NI], F32, tag="rhoi")
        nc.vector.tensor_mul(rhoi, am0[0][0:1], vm0n)
        nc.vector.tensor_scalar(rhoi, rhoi, 1.0, None, op0=ALU.add)
        rhop = st.tile([1, 1], F32, tag="rhop")
        nc.vector.reduce_sum(rhop, rhoi, axis=AX.X)

        # ---- AllReduce: [0]=pad, [1:197]=Jsum by patch, [197]=rho
        arin = P["dr"].tile([1, 256], F32, tag="arin")
        arout = P["dr"].tile([1, 256], F32, tag="arout")
        zpad = st.tile([1, 64], F32, tag="zpad")
        nc.vector.memset(zpad, 0.0)
        nc.sync.dma_start(arin[0:1, 0:128].rearrange("o p -> (o p)")[:, None],
                          jsum[0])
        nc.sync.dma_start(arin[0:1, 128:197].rearrange("o p -> (o p)")[:, None],
                          jsum[1][0:69])
        nc.sync.dma_start(arin[0:1, 197:198], rhop)
        nc.sync.dma_start(arin[0:1, 198:256], zpad[0:1, 0:58])
        nc.gpsimd.collective_compute(
            "AllReduce", ALU.add,
            ins=[arin.opt()], outs=[arout.opt()],
            replica_groups=[list(range(NCORES))])

        jrow = P["dec"].tile([1, 196], F32, tag="jrow")
        nc.sync.dma_start(jrow, arout[0:1, 1:197])
        jcol = []
        for c, (p0, csz) in enumerate(PCHUNKS):
            t = P["dec"].tile([128, 1], F32, tag=f"jcol{c}")
            nc.sync.dma_start(
                t[:csz],
                arout[0:1, 1 + p0:1 + p0 + csz].rearrange("o p -> (o p)")[:, None])
            jcol.append(t)
        rhosum = st.tile([1, 1], F32, tag="rhosum")
        nc.sync.dma_start(rhosum, arout[0:1, 197:198])

        mass = st.tile([1, 1], F32, tag="mass")
        nc.vector.reduce_sum(mass, jrow, axis=AX.X)
        nc.vector.tensor_scalar(mass, mass, 1.0 / BATCH, None, op0=ALU.mult)

        if l >= 1:
            rho = st.tile([1, 1], F32, tag="rho")
            nc.vector.tensor_scalar(rho, rhosum, 1.0 / BATCH, None, op0=ALU.mult)
            et_ = st.tile([1, 1], F32, tag="eta")
            nc.vector.tensor_scalar(et_, self.prev_mass, EPS, None, op0=ALU.add)
            nc.vector.reciprocal(et_, et_)
            nc.vector.tensor_mul(et_, mass, et_)
            r = st.tile([1, 1], F32, tag="krr")
            nc.vector.tensor_mul(r, rho, et_)
            nc.vector.tensor_scalar(r, r, 0.25, 4.0, op0=ALU.max, op1=ALU.min)
            nc.scalar.activation(r, r, AF.Ln)
            nc.scalar.activation(r, r, AF.Exp, scale=-GAMMA)
            qv = st.tile([1, 1], F32, tag="qv")
            nc.vector.tensor_mul(qv, self.n_tok, r)
            cond = st.tile([1, 1], F32, tag="cond")
            nc.vector.tensor_tensor(cond, qv, self.n_tok, ALU.is_lt)
            qm1 = st.tile([1, 1], F32, tag="qm1")
            nc.vector.tensor_scalar(qm1, qv, 1.0, None, op0=ALU.subtract)
            condb = st.tile([128, 1], F32, tag="condb")
            nc.gpsimd.partition_broadcast(condb, cond)
            qm1b = st.tile([128, 1], F32, tag="qm1b")
            nc.gpsimd.partition_broadcast(qm1b, qm1)

            srow = P["dec"].tile([1, 196], F32, tag="srow")
            nc.vector.tensor_scalar(srow, jrow, 1.0, None, op0=ALU.add)
            nc.vector.tensor_mul(srow, srow, self.mask_prow)
            nc.vector.tensor_scalar(srow, srow, 1.0, None, op0=ALU.subtract)
            srowb = P["dec"].tile([128, 196], F32, tag="srowb")
            nc.gpsimd.partition_broadcast(srowb, srow)

            for c, (p0, csz) in enumerate(PCHUNKS):
                scol = P["dec"].tile([128, 1], F32, tag=f"scol{c}")
                nc.vector.tensor_scalar(scol[:csz], jcol[c][:csz], 1.0, None,
                                        op0=ALU.add)
                nc.vector.tensor_mul(scol[:csz], scol[:csz],
                                     self.mask_pcol[c][:csz])
                nc.vector.tensor_scalar(scol[:csz], scol[:csz], 1.0, None,
                                        op0=ALU.subtract)
                cgt = P["dec"].tile([128, 196], F32, tag="cgt")
                nc.vector.tensor_scalar(cgt[:csz], srowb[:csz], scol[:csz], None,
                                        op0=ALU.is_gt)
                rank = P["dec"].tile([128, 1], F32, tag=f"rank{c}")
                nc.vector.reduce_sum(rank[:csz], cgt[:csz], axis=AX.X)
                nc.vector.tensor_scalar(cgt[:csz], srowb[:csz], scol[:csz], None,
                                        op0=ALU.is_equal)
                nc.vector.tensor_mul(cgt[:csz], cgt[:csz], self.tri[c][:csz])
                eqc = P["dec"].tile([128, 1], F32, tag=f"eqc{c}")
                nc.vector.reduce_sum(eqc[:csz], cgt[:csz], axis=AX.X)
                nc.vector.tensor_add(rank[:csz], rank[:csz], eqc[:csz])
                f1 = P["dec"].tile([128, 1], F32, tag=f"f1{c}")
                nc.vector.tensor_scalar(f1[:csz], rank[:csz], 15.0, None,
                                        op0=ALU.is_le)
                f2 = P["dec"].tile([128, 1], F32, tag=f"f2{c}")
                nc.vector.tensor_scalar(f2[:csz], rank[:csz], qm1b[:csz], None,
                                        op0=ALU.is_le)
                nc.vector.tensor_tensor(f1[:csz], f1[:csz], f2[:csz], ALU.max)
                nc.vector.tensor_sub(f1[:csz], f1[:csz], self.mask_pcol[c][:csz])
                nc.vector.tensor_mul(f1[:csz], f1[:csz], condb[:csz])
                nc.vector.tensor_add(self.mask_pcol[c][:csz],
                                     self.mask_pcol[c][:csz], f1[:csz])

            msc = P["dr"].tile([1, 196], F32, tag="msc")
            nc.sync.dma_start(msc[0:1, 0:128].rearrange("o p -> (o p)")[:, None],
                              self.mask_pcol[0])
            nc.sync.dma_start(msc[0:1, 128:196].rearrange("o p -> (o p)")[:, None],
                              self.mask_pcol[1][0:68])
            nc.sync.dma_start(self.mask_prow, msc)
            tmp0 = P["dec"].tile([128, 1], F32, tag="tmp0")
            nc.sync.dma_start(tmp0[1:128],
                              msc[0:1, 0:127].rearrange("o p -> (o p)")[:, None])
            nc.vector.memset(tmp0[0:1], 1.0)
            tmp1 = P["dec"].tile([128, 1], F32, tag="tmp1")
            nc.sync.dma_start(tmp1[0:69],
                              msc[0:1, 127:196].rearrange("o p -> (o p)")[:, None])
            for c, tmp in ((0, tmp0), (1, tmp1)):
                nc.vector.tensor_scalar(self.mbias_tok[c], tmp, 1e9, -1e9,
                                        op0=ALU.mult, op1=ALU.add)
                nc.vector.tensor_copy(self.pmask_tok[c], tmp)
            nc.vector.memset(self.pmask_tok[0][0:1], 0.0)
            nc.vector.reduce_sum(self.n_tok, self.mask_prow, axis=AX.X)

        nc.vector.tensor_copy(self.prev_mass, mass)

    # ---------------- final LN + classifier head ----------------
    def _head(self):
        nc, P, ap = self.nc, self.P, self.ap
        cls = [t.rearrange("p (a b) -> p a b", a=NI)[:, :, 0] for t in self.xT]
        ps_s = P["psR"].tile([1, 512], F32, tag="psR")
        ps_q = P["psR"].tile([1, 512], F32, tag="psR")
        for d in range(KD):
            sq = P["sq"].tile([128, 512], F32, tag="sq")
            nc.scalar.activation(sq[:, :NI], cls[d], AF.Square)
            nc.tensor.matmul(ps_s[:, :NI], self.ones_col, cls[d],
                             start=(d == 0), stop=(d == KD - 1))
            nc.tensor.matmul(ps_q[:, :NI], self.ones_col, sq[:, :NI],
                             start=(d == 0), stop=(d == KD - 1))
        mu = P["st"].tile([1, NI], F32, tag="h_mu")
        nc.scalar.activation(mu, ps_s[:, :NI], AF.Copy, scale=1.0 / D)
        var = P["st"].tile([1, NI], F32, tag="h_var")
        nc.scalar.activation(var, ps_q[:, :NI], AF.Copy, scale=1.0 / D)
        tmp = P["st"].tile([1, NI], F32, tag="h_tmp")
        nc.vector.tensor_mul(tmp, mu, mu)
        nc.vector.tensor_sub(var, var, tmp)
        nc.scalar.activation(var, var, AF.Sqrt, bias=self.eps_row)
        nc.vector.reciprocal(var, var)                       # rstd
        crow = P["st"].tile([1, NI], F32, tag="h_crow")
        nc.vector.tensor_mul(crow, mu, var)
        rstdb = P["st"].tile([128, NI], F32, tag="h_rstdb")
        nc.gpsimd.partition_broadcast(rstdb, var)
        crowb = P["st"].tile([128, NI], F32, tag="h_crowb")
        nc.gpsimd.partition_broadcast(crowb, crow)

        nw = P["bia"].tile([128, KD], F32, tag="lnw")
        nc.sync.dma_start(nw, _col(ap["normw"]))
        nb = P["bia"].tile([128, KD], F32, tag="lnb")
        nc.sync.dma_start(nb, _col(ap["normb"]))
        headb = P["bia"].tile([1, NCLS], F32, tag="vb")
        nc.sync.dma_start(headb, ap["headb"][None, :])

        xcl = P["st"].tile([128, KD, NI], F32, tag="xcl")
        for d in range(KD):
            nc.vector.tensor_mul(xcl[:, d, :], cls[d], rstdb)
            nc.vector.tensor_sub(xcl[:, d, :], xcl[:, d, :], crowb)
            nc.vector.tensor_scalar(xcl[:, d, :], xcl[:, d, :], nw[:, d:d + 1],
                                    nb[:, d:d + 1], op0=ALU.mult, op1=ALU.add)

        outsb = P["st"].tile([NI, NCLS], F32, tag="outsb")
        for n0, nsz in [(0, 512), (512, 488)]:
            ps = P["psB"].tile([128, 512], F32, tag="psB")
            for k in range(KD):
                wt = P["vw"].tile([128, 788], F32, tag=f"vw{k}")
                nc.sync.dma_start(wt[:, :nsz],
                                  ap["headw"][k * 128:(k + 1) * 128, n0:n0 + nsz])
                nc.tensor.matmul(ps[:NI, :nsz], xcl[:, k, :], wt[:, :nsz],
                                 start=(k == 0), stop=False)
            nc.tensor.matmul(ps[:NI, :nsz], self.ones_row[0:1, 0:NI],
                             headb[0:1, n0:n0 + nsz], start=False, stop=True)
            nc.vector.tensor_copy(outsb[:, n0:n0 + nsz], ps[:NI, :nsz])
        nc.sync.dma_start(self.out_ext.ap(), outsb)


# ===================== host side =====================

_NC_CACHE = {}


def _im2col(x):
    b = x.shape[0]
    x = x.reshape(b, 3, 14, PATCH, 14, PATCH)
    x = x.transpose(0, 2, 4, 1, 3, 5)
    return x.reshape(b, NPATCH, 3 * PATCH * PATCH)


def prep_inputs(x, patch_w, patch_b, cls_token, pos_embed, ln1_w, ln1_b,
                qkv_w, qkv_b, proj_w, proj_b, ln2_w, ln2_b, fc1_w, fc1_b,
                fc2_w, fc2_b, norm_w, norm_b, head_w, head_b):
    import ml_dtypes
    f32 = np.float32
    asf = lambda a: np.ascontiguousarray(np.asarray(a), dtype=f32)
    x = asf(x); patch_w = asf(patch_w); patch_b = asf(patch_b)
    cls_token = asf(cls_token); pos_embed = asf(pos_embed)
    qkv_w = asf(qkv_w).copy(); qkv_b = asf(qkv_b).copy()
    qkv_w[:, :, :D] *= f32(0.125)      # fold attention scale into q (exact)
    qkv_b[:, :D] *= f32(0.125)
    vmw = np.ascontiguousarray(qkv_w[:, :, 2 * D:].reshape(DEPTH, D, NH, DH)
                               .mean(axis=2, dtype=f32))
    vmb = np.ascontiguousarray(qkv_b[:, 2 * D:].reshape(DEPTH, NH, DH)
                               .mean(axis=1, dtype=f32))
    embw = np.ascontiguousarray(patch_w.reshape(D, 3 * PATCH * PATCH).T)
    posT = np.zeros((D, TT), f32)
    for i in range(NI):
        posT[:, i * TPI] = cls_token[0, 0] + pos_embed[0, 0]
        posT[:, i * TPI + 1:(i + 1) * TPI] = (pos_embed[0, 1:] + patch_b[None, :]).T

    common = dict(
        posT=posT, embw=embw,
        ln1w=asf(ln1_w), ln1b=asf(ln1_b), qkvw=qkv_w, qkvb=qkv_b,
        vmw=vmw, vmb=vmb, projw=asf(proj_w), projb=asf(proj_b),
        ln2w=asf(ln2_w), ln2b=asf(ln2_b),
        fc1w=asf(fc1_w), fc1b=asf(fc1_b),
        fc2wb=np.ascontiguousarray(asf(fc2_w).astype(ml_dtypes.bfloat16)),
        fc2b=asf(fc2_b),
        normw=asf(norm_w), normb=asf(norm_b),
        headw=asf(head_w), headb=asf(head_b),
    )
    pm = _im2col(x)
    in_maps = []
    for core in range(NCORES):
        xpT = np.zeros((D, TT), f32)
        for i in range(NI):
            xpT[:, i * TPI + 1:(i + 1) * TPI] = pm[core * NI + i].T
        in_maps.append({"xpT": xpT, **common})
    return in_maps


def get_nc():
    if "nc" not in _NC_CACHE:
        _NC_CACHE["nc"] = build()
    return _NC_CACHE["nc"]


def kernel(**inputs):
    from concourse.bass_utils import run_bass_kernel_spmd
    in_maps = prep_inputs(**inputs)
    nc = get_nc()
    res = run_bass_kernel_spmd(nc, in_maps, core_ids=list(range(NCORES)))
    return np.concatenate([res.results[i]["out"] for i in range(NCORES)], axis=0)
